# revision 29
# baseline (speedup 1.0000x reference)
"""Trainium2 Bass kernel for MQA causal attention with null token.

Problem (reference.py):
  b=4, n=2048, dim=1024, HEADS=16, DIM_HEAD=64
  q  = (x @ Wq).reshape(b,n,16,64).transpose -> [b,h,n,64] * 64**-0.5
  kv = x @ Wkv -> [b,n,64]; prepend null -> [b,2049,64]
  sim = q @ kv^T  (causal: query i sees kv cols 0..i+1)
  out = softmax(sim) @ kv -> concat heads -> @ Wout

Sharding: 8 cores = batch(4) x head-half(2). Each core handles one batch
element and 8 heads, producing a partial out-projection; host adds the two
half-head partials per batch.

Device algorithm (per core), all matmuls fp16 inputs w/ fp32 PSUM accumulate.
Software-pipelined per q-block (512 queries):
  project QT into zero-padded per-parity tiles (qt2e rows 0:64 = even head,
    rows 64:128 = 0; qt2o mirrored) + KVT2 block (kv duplicated on 128 rows)
  build KV_aug chunks [128,65] (PE transpose of KVT2 + ones column)
  scores TRANSPOSED (k on partitions): ST[c] = KVT2_chunk contract padded QT
    -> K=128 matmuls (measured 2x faster per column than K=64 on TRN2);
    diagonal chunks narrowed to the causally-visible column range
  exp on ACT (PSUM->SBUF fp16), causal mask multiply on diagonal chunks (DVE)
  PV: OT_raw[65,512] += KV_aug_chunk^T @ expST (row 64 = softmax denominator),
    emitted `lag` chunks behind the ST/exp stream so the in-order PE queue
    never head-blocks on an exp
  proj of the next q-block / out-proj of the previous one are emitted as
    small filler units inside the attention chunk stream to fill PE bubbles
  normalize (deferred into the next pair's stream): DVE reciprocal into row
    64 of a zeroed [128,512] tile -> all-ones K=128 matmul broadcasts 1/den
    -> DVE multiply -> AT
  out-proj: AT^T-contract @ Wout -> fp32 -> HBM; host adds half-head partials
"""

import sys

for _p in ("/opt/trn_rl_repo",):
    if _p not in sys.path:
        sys.path.insert(0, _p)

import numpy as np

HEADS = 16
DH = 64
B = 4
N = 2048
DIM = 1024
NQB = 4          # q blocks of 512 per head
QB = 512
KTOT = N + 1     # 2049 kv positions (null at 0)

_PROGRAM_CACHE = {}


def _build_program(expst_bufs=10, pools="v5", order="inter", edge_early=True,
                   bcast_via="pad", ablate=(), reps=0, recip_impl="dve",
                   bcast_copy="vector", qpad=1, maskeng="dve", qcopy="dve",
                   ocopy="dve", lag=2, fin_inline=0, kvpad=1):
    import concourse.bacc as bacc
    import concourse.tile as tile
    import concourse.mybir as mybir

    f16 = mybir.dt.float16
    f32 = mybir.dt.float32
    EXP = mybir.ActivationFunctionType.Exp
    LN = getattr(mybir.ActivationFunctionType, "Ln",
                 getattr(mybir.ActivationFunctionType, "Log", None))

    nc = bacc.Bacc("TRN2", debug=False, num_devices=8)

    xt_d = nc.dram_tensor("xt", [DIM, N], f16, kind="ExternalInput").ap()
    wq_d = nc.dram_tensor("wq", [DIM, 512], f16, kind="ExternalInput").ap()
    wkv2_d = nc.dram_tensor("wkv2", [DIM, 128], f16, kind="ExternalInput").ap()
    nullkv2_d = nc.dram_tensor("nullkv2", [128, 1], f16, kind="ExternalInput").ap()
    wout_d = nc.dram_tensor("wout", [512, DIM], f16, kind="ExternalInput").ap()
    masks_d = nc.dram_tensor("masks", [128, 4 * QB], f16, kind="ExternalInput").ap()
    ident_d = nc.dram_tensor("ident", [128, 128], f16, kind="ExternalInput").ap()
    out_d = nc.dram_tensor("out", [N, DIM], f32, kind="ExternalOutput").ap()

    # diagonal chunk t: visible cols j >= j0; (j0, width), col offset in tile
    DIAG = []
    off = [0, 0]
    for t in range(4):
        j0 = 0 if t == 0 else 128 * t - 1
        w = QB - j0
        grp = t // 2
        DIAG.append((t, j0, w, grp, off[grp]))
        off[grp] += w

    with tile.TileContext(nc) as tc:
        from contextlib import ExitStack

        with ExitStack() as ctx:
            consts = ctx.enter_context(tc.tile_pool(name="consts", bufs=1))
            work = ctx.enter_context(tc.tile_pool(name="work", bufs=expst_bufs))
            small = ctx.enter_context(tc.tile_pool(name="small", bufs=2))
            # PSUM: st pool (2x2 banks, shared with projection tiles),
            # ot pool (2x1), misc pool (2x1: transpose / edge / rep tiles)
            st_ps = ctx.enter_context(tc.tile_pool(
                name="st_ps",
                bufs=(3 if pools == "v5" else 2 if pools == "v6" else
                      4 if order in ("phase2", "phase3", "inter") else 2),
                space="PSUM"))
            if pools == "v5":
                pj_ps = ctx.enter_context(tc.tile_pool(name="pj_ps", bufs=2, space="PSUM"))
                ot_ps = ctx.enter_context(tc.tile_pool(name="ot_ps", bufs=3, space="PSUM"))
                mi_ps = pj_ps
            elif pools == "v6":
                pj_ps = ctx.enter_context(tc.tile_pool(name="pj_ps", bufs=3, space="PSUM"))
                ot_ps = ctx.enter_context(tc.tile_pool(name="ot_ps", bufs=3, space="PSUM"))
                mi_ps = pj_ps
            elif pools == "v1":
                pj_ps = ctx.enter_context(tc.tile_pool(name="pj_ps", bufs=2, space="PSUM"))
                ot_ps = ctx.enter_context(tc.tile_pool(name="ot_ps", bufs=1, space="PSUM"))
                mi_ps = ctx.enter_context(tc.tile_pool(name="mi_ps", bufs=1, space="PSUM"))
            elif pools == "v2":
                pj_ps = ctx.enter_context(tc.tile_pool(name="pj_ps", bufs=2, space="PSUM"))
                ot_ps = ctx.enter_context(tc.tile_pool(name="ot_ps", bufs=2, space="PSUM"))
                mi_ps = pj_ps
            elif pools == "v4":
                pj_ps = ctx.enter_context(tc.tile_pool(name="pj_ps", bufs=1, space="PSUM"))
                ot_ps = ctx.enter_context(tc.tile_pool(name="ot_ps", bufs=3, space="PSUM"))
                mi_ps = pj_ps
            else:  # v3
                pj_ps = ctx.enter_context(tc.tile_pool(name="pj_ps", bufs=1, space="PSUM"))
                ot_ps = ctx.enter_context(tc.tile_pool(name="ot_ps", bufs=2, space="PSUM"))
                mi_ps = ctx.enter_context(tc.tile_pool(name="mi_ps", bufs=1, space="PSUM"))

            # ---- persistent SBUF tiles ----
            xt_sb = consts.tile([128, 8, N], f16, tag="xt")
            wq_sb = consts.tile([128, 8, 512], f16, tag="wq")
            wkv2_sb = consts.tile([128, 8, 128], f16, tag="wkv2")
            wout_sb = consts.tile([128, 4, DIM], f16, tag="wout")
            masks_sb = consts.tile([128, 4 * QB], f16, tag="masks")
            ident_sb = consts.tile([128, 128], f16, tag="ident")
            kvt2_sb = consts.tile([128, KTOT], f16, tag="kvt2")
            ones_sb = consts.tile([128, 64], f16, tag="ones")
            KVS = 128 if kvpad else 65
            kvaug_sb = consts.tile([128, 17 * KVS], f16, tag="kvaug")
            if kvpad:
                nc.vector.memset(kvaug_sb, 0.0)
            qt2_sb = consts.tile([128, 4, N], f16, tag="qt2")
            at_sb = consts.tile([128, 4, N], f16, tag="at")
            if qpad:
                # zero-padded per-parity q: qt2e rows 64:128 stay 0,
                # qt2o rows 0:64 stay 0 -> K=128 score matmuls
                qt2e_sb = consts.tile([128, 4, N], f16, tag="qt2e")
                qt2o_sb = consts.tile([128, 4, N], f16, tag="qt2o")
                nc.vector.memset(qt2e_sb[64:128], 0.0)
                nc.vector.memset(qt2o_sb[0:64], 0.0)
                qt2p = (qt2e_sb, qt2o_sb)
            if bcast_via == "pad":
                # zeroed [128,512] tiles; row 64 gets 1/den, then an
                # all-ones K=128 matmul broadcasts it to 64 partitions
                rp_sb = [consts.tile([128, 512], f16, tag=f"rp{i}",
                                     name=f"rp{i}") for i in range(4)]
                for t in rp_sb:
                    nc.vector.memset(t, 0.0)

            nc.vector.memset(ones_sb, 1.0)
            xt_r = xt_d.rearrange("(d p) t -> p d t", p=128)
            for d in range(8):
                nc.sync.dma_start(out=xt_sb[:, d, :], in_=xt_r[:, d, :])
            nc.sync.dma_start(out=wq_sb, in_=wq_d.rearrange("(d p) m -> p d m", p=128))
            nc.sync.dma_start(
                out=wkv2_sb, in_=wkv2_d.rearrange("(d p) m -> p d m", p=128)
            )
            nc.sync.dma_start(
                out=wout_sb, in_=wout_d.rearrange("(f p) o -> p f o", p=128)
            )
            nc.sync.dma_start(out=masks_sb, in_=masks_d)
            nc.sync.dma_start(out=ident_sb, in_=ident_d)
            nc.sync.dma_start(out=kvt2_sb[:, 0:1], in_=nullkv2_d)

            qc_eng = {"dve": nc.vector.tensor_copy, "act": nc.scalar.copy}[
                "act" if qcopy == "act" else "dve"]

            def emit_proj(qb):
                qs = slice(qb * QB, (qb + 1) * QB)
                for pair in range(4):
                    qp = pj_ps.tile([128, 512], f32, tag="mi")
                    for d in range(8):
                        nc.tensor.matmul(
                            qp[:, 0:512],
                            lhsT=wq_sb[:, d, pair * 128:(pair + 1) * 128],
                            rhs=xt_sb[:, d, qs],
                            start=(d == 0),
                            stop=(d == 7),
                        )
                    if qpad:
                        qc_eng(qt2e_sb[0:64, pair, qs], qp[0:64, 0:512])
                        qc_eng(qt2o_sb[64:128, pair, qs], qp[64:128, 0:512])
                    else:
                        nc.scalar.copy(qt2_sb[:, pair, qs], qp[:, 0:512])
                kp = pj_ps.tile([128, 512], f32, tag="mi")
                for d in range(8):
                    nc.tensor.matmul(
                        kp[:, 0:512],
                        lhsT=wkv2_sb[:, d, :],
                        rhs=xt_sb[:, d, qs],
                        start=(d == 0),
                        stop=(d == 7),
                    )
                nc.vector.tensor_copy(kvt2_sb[:, 1 + qb * 512:513 + qb * 512], kp[:, 0:512])

            def emit_kvaug(qb):
                for c in range(4 * qb, 4 * qb + 4):
                    tp = mi_ps.tile([128, 64], f16, tag="mi")
                    nc.tensor.transpose(
                        tp, kvt2_sb[0:64, c * 128:(c + 1) * 128], ident_sb[0:64, 0:64]
                    )
                    nc.vector.tensor_copy(kvaug_sb[:, c * KVS:c * KVS + 64], tp)
                    nc.vector.memset(kvaug_sb[:, c * KVS + 64:c * KVS + 65], 1.0)
                cE = 4 * qb + 4
                kE = 128 * cE
                tpe = mi_ps.tile([128, 64], f16, tag="mi")
                nc.tensor.transpose(
                    tpe[0:1, :], kvt2_sb[0:64, kE:kE + 1], ident_sb[0:64, 0:64]
                )
                nc.vector.tensor_copy(kvaug_sb[0:1, cE * KVS:cE * KVS + 64], tpe[0:1, :])
                nc.vector.memset(kvaug_sb[0:1, cE * KVS + 64:cE * KVS + 65], 1.0)

            def emit_attn(pair, qb, parity):
                qs = slice(qb * QB, (qb + 1) * QB)
                cE = 4 * qb + 4
                kE = 128 * cE
                p0 = 64 * parity
                ot = ot_ps.tile([65, 512], f32, tag="ot")

                def st_mm(dst, c, j0, w):
                    nc.tensor.matmul(
                        dst,
                        lhsT=kvt2_sb[p0:p0 + 64, c * 128:(c + 1) * 128],
                        rhs=qt2_sb[p0:p0 + 64, pair, qb * QB + j0:(qb + 1) * QB],
                        start=True,
                        stop=True,
                    )

                def pv_mm(c, rhs_ap, j0):
                    nc.tensor.matmul(
                        ot[:, j0:512],
                        lhsT=kvaug_sb[:, c * 65:c * 65 + 65],
                        rhs=rhs_ap,
                        start=(c == 0),
                        stop=False,
                    )

                if edge_early:
                    es = mi_ps.tile([128, 64], f32, tag="mi")
                    nc.tensor.matmul(
                        es[0:1, 0:1],
                        lhsT=kvt2_sb[p0:p0 + 64, kE:kE + 1],
                        rhs=qt2_sb[p0:p0 + 64, pair,
                                   qb * QB + 511:qb * QB + 512],
                        start=True,
                        stop=True,
                    )
                    ee = small.tile([1, 1], f16, tag="edge_sb")
                    nc.scalar.activation(ee, es[0:1, 0:1], EXP)

                for g in range(qb * 2):
                    st = st_ps.tile([128, 1024], f32, tag="st")
                    for i in range(2):
                        c = 2 * g + i
                        st_mm(st[:, i * 512:(i + 1) * 512], c, 0, QB)
                    expst = work.tile([128, 1024], f16, tag="expst")
                    if "exp" in ablate:
                        nc.scalar.activation(expst[0:1, 0:1], st[0:1, 0:1], EXP)
                    else:
                        nc.scalar.activation(expst, st, EXP)
                    for i in range(2):
                        c = 2 * g + i
                        pv_mm(c, expst[:, i * 512:(i + 1) * 512], 0)

                for grp, gw in ((0, 897), (1, 386)):
                    st = st_ps.tile([128, 1024], f32, tag="st")
                    for t, j0, w, g_, off in DIAG:
                        if g_ != grp:
                            continue
                        st_mm(st[:, off:off + w], 4 * qb + t, j0, w)
                    expst = work.tile([128, 1024], f16, tag="expst")
                    if "exp" in ablate:
                        nc.scalar.activation(expst[0:1, 0:1], st[0:1, 0:1], EXP)
                    else:
                        nc.scalar.activation(expst[:, 0:gw], st[:, 0:gw], EXP)
                    for t, j0, w, g_, off in DIAG:
                        if g_ != grp:
                            continue
                        if "mask" not in ablate:
                            nc.vector.tensor_mul(
                                expst[:, off:off + w],
                                expst[:, off:off + w],
                                masks_sb[:, t * QB + j0:(t + 1) * QB],
                            )
                        pv_mm(4 * qb + t, expst[:, off:off + w], j0)

                if not edge_early:
                    es = mi_ps.tile([128, 64], f32, tag="mi")
                    nc.tensor.matmul(
                        es[0:1, 0:1],
                        lhsT=kvt2_sb[p0:p0 + 64, kE:kE + 1],
                        rhs=qt2_sb[p0:p0 + 64, pair,
                                   qb * QB + 511:qb * QB + 512],
                        start=True,
                        stop=True,
                    )
                    ee = small.tile([1, 1], f16, tag="edge_sb")
                    nc.scalar.activation(ee, es[0:1, 0:1], EXP)
                nc.tensor.matmul(
                    ot[:, 511:512],
                    lhsT=kvaug_sb[0:1, cE * 65:cE * 65 + 65],
                    rhs=ee,
                    start=False,
                    stop=True,
                )

                recip = small.tile([65, 512], f16, tag="recip")
                if "recip" in ablate:
                    nc.vector.tensor_copy(recip[64:65, :], ot[64:65, :])
                elif recip_impl == "gpsimd":
                    den = small.tile([65, 512], f32, tag="lnd")
                    nc.vector.tensor_copy(den[64:65, :], ot[64:65, :])
                    with nc.allow_low_precision(reason="softmax recip f16"):
                        nc.gpsimd.reciprocal(recip[64:65, :], den[64:65, :])
                elif recip_impl == "lnexp":
                    lnd = small.tile([65, 512], f32, tag="lnd")
                    nc.scalar.activation(lnd[64:65, :], ot[64:65, :], LN)
                    nc.scalar.activation(recip[64:65, :], lnd[64:65, :], EXP,
                                         scale=-1.0)
                else:
                    with nc.allow_low_precision(reason="softmax recip f16"):
                        nc.vector.reciprocal(recip[64:65, :], ot[64:65, :])
                if bcast_via == "pe":
                    rep = mi_ps.tile([65, 512], f32, tag="mi")
                    nc.tensor.matmul(
                        rep[0:64, :],
                        lhsT=ones_sb[64:65, :],
                        rhs=recip[64:65, :],
                        start=True,
                        stop=True,
                    )
                    bcast = small.tile([64, 512], f32, tag="bcast")
                    if bcast_copy == "scalar":
                        nc.scalar.copy(bcast, rep[0:64, :])
                    else:
                        nc.vector.tensor_copy(bcast, rep[0:64, :])
                else:
                    import concourse.bass as _bass
                    bcast = small.tile([64, 512], f16, tag="bcast")
                    rsl = recip[64:65, :]
                    bsrc = _bass.AP(
                        tensor=rsl.tensor, offset=rsl.offset,
                        ap=[[0, 64]] + [list(p) for p in list(rsl.ap)[1:]],
                    )
                    nc.sync.dma_start(out=bcast, in_=bsrc)
                if "norm" in ablate:
                    nc.vector.tensor_copy(at_sb[0:64, pair, qs], ot[0:64, :])
                elif parity == 0:
                    nc.vector.tensor_mul(
                        at_sb[0:64, pair, qs], ot[0:64, :], bcast
                    )
                else:
                    stg = small.tile([64, 512], f16, tag="stg")
                    nc.vector.tensor_mul(stg, ot[0:64, :], bcast)
                    nc.sync.dma_start(out=at_sb[64:128, pair, qs], in_=stg)

            def emit_attn2(pair, qb):
                qs = slice(qb * QB, (qb + 1) * QB)
                cE = 4 * qb + 4
                kE = 128 * cE
                ots = {}
                ees = {}
                for parity in range(2):
                    p0 = 64 * parity
                    ot_t = ot_ps.tile([65, 512], f32, tag="ot")
                    ots[parity] = ot_t
                    es = mi_ps.tile([128, 64], f32, tag="mi")
                    nc.tensor.matmul(
                        es[0:1, 0:1],
                        lhsT=kvt2_sb[p0:p0 + 64, kE:kE + 1],
                        rhs=qt2_sb[p0:p0 + 64, pair,
                                   qb * QB + 511:qb * QB + 512],
                        start=True,
                        stop=True,
                    )
                    ee_t = small.tile([1, 1], f16, tag="edge_sb")
                    ees[parity] = ee_t
                    nc.scalar.activation(ee_t, es[0:1, 0:1], EXP)

                def chunk_info(c):
                    t = c - 4 * qb
                    if t >= 0:
                        j0 = 0 if t == 0 else 128 * t - 1
                        return t, j0, QB - j0
                    return -1, 0, QB

                for c in range(4 * qb + 4):
                    t, j0, w = chunk_info(c)
                    sts = {}
                    for parity in range(2):
                        p0 = 64 * parity
                        st = st_ps.tile([128, 512], f32, tag="st")
                        nc.tensor.matmul(
                            st[:, 0:w],
                            lhsT=kvt2_sb[p0:p0 + 64, c * 128:(c + 1) * 128],
                            rhs=qt2_sb[p0:p0 + 64, pair,
                                       qb * QB + j0:(qb + 1) * QB],
                            start=True,
                            stop=True,
                        )
                        sts[parity] = st
                    for parity in range(2):
                        st = sts[parity]
                        expst = work.tile([128, 512], f16, tag="expst")
                        if "exp" in ablate:
                            nc.scalar.activation(expst[0:1, 0:1], st[0:1, 0:1], EXP)
                        else:
                            nc.scalar.activation(expst[:, 0:w], st[:, 0:w], EXP)
                        if t >= 0 and "mask" not in ablate:
                            nc.vector.tensor_mul(
                                expst[:, 0:w],
                                expst[:, 0:w],
                                masks_sb[:, t * QB + j0:(t + 1) * QB],
                            )
                        nc.tensor.matmul(
                            ots[parity][:, j0:512],
                            lhsT=kvaug_sb[:, c * 65:c * 65 + 65],
                            rhs=expst[:, 0:w],
                            start=(c == 0),
                            stop=False,
                        )
                for parity in range(2):
                    p0 = 64 * parity
                    ot = ots[parity]
                    nc.tensor.matmul(
                        ot[:, 511:512],
                        lhsT=kvaug_sb[0:1, cE * 65:cE * 65 + 65],
                        rhs=ees[parity],
                        start=False,
                        stop=True,
                    )
                    recip = small.tile([65, 512], f16, tag="recip")
                    if "recip" in ablate:
                        nc.vector.tensor_copy(recip[64:65, :], ot[64:65, :])
                    elif recip_impl == "gpsimd":
                        den = small.tile([65, 512], f32, tag="lnd")
                        nc.vector.tensor_copy(den[64:65, :], ot[64:65, :])
                        with nc.allow_low_precision(reason="softmax recip f16"):
                            nc.gpsimd.reciprocal(recip[64:65, :], den[64:65, :])
                    elif recip_impl == "lnexp":
                        lnd = small.tile([65, 512], f32, tag="lnd")
                        nc.scalar.activation(lnd[64:65, :], ot[64:65, :], LN)
                        nc.scalar.activation(recip[64:65, :], lnd[64:65, :], EXP,
                                             scale=-1.0)
                    else:
                        with nc.allow_low_precision(reason="softmax recip f16"):
                            nc.vector.reciprocal(recip[64:65, :], ot[64:65, :])
                    rep = mi_ps.tile([65, 512], f32, tag="mi")
                    nc.tensor.matmul(
                        rep[0:64, :],
                        lhsT=ones_sb[64:65, :],
                        rhs=recip[64:65, :],
                        start=True,
                        stop=True,
                    )
                    bcast = small.tile([64, 512], f32, tag="bcast")
                    if bcast_copy == "scalar":
                        nc.scalar.copy(bcast, rep[0:64, :])
                    else:
                        nc.vector.tensor_copy(bcast, rep[0:64, :])
                    if "norm" in ablate:
                        nc.vector.tensor_copy(at_sb[0:64, pair, qs], ot[0:64, :])
                    elif parity == 0:
                        nc.vector.tensor_mul(
                            at_sb[0:64, pair, qs], ot[0:64, :], bcast
                        )
                    else:
                        stg = small.tile([64, 512], f16, tag="stg")
                        nc.vector.tensor_mul(stg, ot[0:64, :], bcast)
                        nc.sync.dma_start(out=at_sb[64:128, pair, qs], in_=stg)

            def emit_attn3(pair, qb, fillers=None):
                # Software-pipelined attention for one (pair, q-block):
                # K=128 score matmuls via zero-padded per-parity q tiles;
                # PV matmuls lag the ST/exp stream by `lag` chunks so the
                # in-order PE queue never head-blocks on an exp; the
                # normalize tail is returned as two closures for the caller
                # to drain inside the NEXT pair's stream.
                qs = slice(qb * QB, (qb + 1) * QB)
                cE = 4 * qb + 4
                kE = 128 * cE
                mask_mul = (nc.gpsimd.tensor_mul if maskeng == "pool"
                            else nc.vector.tensor_mul)
                ots = {}
                for parity in range(2):
                    ots[parity] = ot_ps.tile([128 if kvpad else 65, 512], f32,
                                             tag="ot", name="ot3")

                def chunk_info(c):
                    t = c - 4 * qb
                    if t >= 0:
                        j0 = 0 if t == 0 else 128 * t - 1
                        return t, j0, QB - j0
                    return -1, 0, QB

                pend = []

                def pv_flush(n_keep):
                    while len(pend) > n_keep:
                        parity, c, j0, w, expst = pend.pop(0)
                        nc.tensor.matmul(
                            ots[parity][:, j0:512],
                            lhsT=kvaug_sb[:, c * KVS:c * KVS + KVS],
                            rhs=expst[:, 0:w],
                            start=(c == 0),
                            stop=False,
                        )

                ees = {}
                for c in range(4 * qb + 4):
                    t, j0, w = chunk_info(c)
                    sts = {}
                    for parity in range(2):
                        st = st_ps.tile([128, 512], f32, tag="st")
                        nc.tensor.matmul(
                            st[:, 0:w],
                            lhsT=kvt2_sb[:, c * 128:(c + 1) * 128],
                            rhs=qt2p[parity][:, pair,
                                             qb * QB + j0:(qb + 1) * QB],
                            start=True,
                            stop=True,
                        )
                        sts[parity] = st
                    if c == 0:
                        # null-edge score for the last query; early so the
                        # tiny ACT exp clears the queue long before the tail
                        for parity in range(2):
                            es = mi_ps.tile([128, 64], f32, tag="mi",
                                            name="es3")
                            nc.tensor.matmul(
                                es[0:1, 0:1],
                                lhsT=kvt2_sb[:, kE:kE + 1],
                                rhs=qt2p[parity][:, pair,
                                                 qb * QB + 511:qb * QB + 512],
                                start=True,
                                stop=True,
                            )
                            ee_t = small.tile([1, 1], f16, tag="edge_sb")
                            ees[parity] = ee_t
                            nc.scalar.activation(ee_t, es[0:1, 0:1], EXP)
                    for parity in range(2):
                        st = sts[parity]
                        expst = work.tile([128, 512], f16, tag="expst")
                        if "exp" in ablate:
                            nc.scalar.activation(expst[0:1, 0:1], st[0:1, 0:1], EXP)
                        else:
                            nc.scalar.activation(expst[:, 0:w], st[:, 0:w], EXP)
                        if t >= 0 and "mask" not in ablate:
                            mask_mul(
                                expst[:, 0:w],
                                expst[:, 0:w],
                                masks_sb[:, t * QB + j0:(t + 1) * QB],
                            )
                        pend.append((parity, c, j0, w, expst))
                    pv_flush(2 * lag)
                    if fillers:
                        n_left = fillers[0]
                        fillers[0] = max(0, n_left - 1)
                        take = -(-len(fillers[1]) // max(1, n_left))
                        for _ in range(min(take, len(fillers[1]))):
                            fillers[1].pop(0)()
                pv_flush(0)

                def fin1():
                    for parity in range(2):
                        nc.tensor.matmul(
                            ots[parity][0:65, 511:512],
                            lhsT=kvaug_sb[0:1, cE * KVS:cE * KVS + 65],
                            rhs=ees[parity],
                            start=False,
                            stop=True,
                        )
                    for parity in range(2):
                        idx = ((qb * 4 + pair) * 2 + parity) % 4
                        rp = rp_sb[idx]
                        with nc.allow_low_precision(reason="softmax recip"):
                            nc.vector.reciprocal(rp[64:65, :],
                                                 ots[parity][64:65, :])

                def fin2():
                    for parity in range(2):
                        ot = ots[parity]
                        idx = ((qb * 4 + pair) * 2 + parity) % 4
                        rp = rp_sb[idx]
                        rep = mi_ps.tile([65, 512], f32, tag="mi", name="rep")
                        nc.tensor.matmul(
                            rep[0:64, :],
                            lhsT=ones_sb[:, 0:64],
                            rhs=rp[:, 0:512],
                            start=True,
                            stop=True,
                        )
                        bcast = small.tile([64, 512], f32, tag="bcast")
                        nc.vector.tensor_copy(bcast, rep[0:64, :])
                        if parity == 0:
                            nc.vector.tensor_mul(
                                at_sb[0:64, pair, qs], ot[0:64, :], bcast
                            )
                        else:
                            stg = small.tile([64, 512], f16, tag="stg")
                            nc.vector.tensor_mul(stg, ot[0:64, :], bcast)
                            nc.sync.dma_start(out=at_sb[64:128, pair, qs],
                                              in_=stg)

                return fin1, fin2

            def emit_outproj(m):
                ms = slice(m * 128, (m + 1) * 128)
                for nn in range(2):
                    os_ = slice(nn * 512, (nn + 1) * 512)
                    op = pj_ps.tile([128, 512], f32, tag="mi")
                    for fc in range(4):
                        nc.tensor.matmul(
                            op[:, 0:512],
                            lhsT=at_sb[:, fc, ms],
                            rhs=wout_sb[:, fc, os_],
                            start=(fc == 0),
                            stop=(fc == 3),
                        )
                    ost = work.tile([128, 512], f32, tag="ost")
                    if ocopy == "dve":
                        nc.vector.tensor_copy(ost, op[:, 0:512])
                    else:
                        nc.scalar.copy(ost, op[:, 0:512])
                    nc.sync.dma_start(out=out_d[ms, os_], in_=ost)

            def proj_units(qb):
                # emit_proj(qb) split into ~4-matmul filler units
                qs = slice(qb * QB, (qb + 1) * QB)
                units = []

                def pair_unit(pair, half, hold={}):
                    def f():
                        if half == 0:
                            hold[pair] = pj_ps.tile([128, 512], f32, tag="mi", name="qpu")
                        qp = hold[pair]
                        for d in range(4 * half, 4 * half + 4):
                            nc.tensor.matmul(
                                qp[:, 0:512],
                                lhsT=wq_sb[:, d, pair * 128:(pair + 1) * 128],
                                rhs=xt_sb[:, d, qs],
                                start=(d == 0),
                                stop=(d == 7),
                            )
                        if half == 1:
                            qc_eng(qt2e_sb[0:64, pair, qs], qp[0:64, 0:512])
                            qc_eng(qt2o_sb[64:128, pair, qs],
                                   qp[64:128, 0:512])
                            del hold[pair]
                    return f

                def kv_unit(half, hold={}):
                    def f():
                        if half == 0:
                            hold[0] = pj_ps.tile([128, 512], f32, tag="mi", name="kpu")
                        kp = hold[0]
                        for d in range(4 * half, 4 * half + 4):
                            nc.tensor.matmul(
                                kp[:, 0:512],
                                lhsT=wkv2_sb[:, d, :],
                                rhs=xt_sb[:, d, qs],
                                start=(d == 0),
                                stop=(d == 7),
                            )
                        if half == 1:
                            nc.vector.tensor_copy(
                                kvt2_sb[:, 1 + qb * 512:513 + qb * 512],
                                kp[:, 0:512],
                            )
                            del hold[0]
                    return f

                for pair in range(4):
                    hold = {}
                    units.append(pair_unit(pair, 0, hold))
                    units.append(pair_unit(pair, 1, hold))
                hold = {}
                units.append(kv_unit(0, hold))
                units.append(kv_unit(1, hold))
                return units

            def outproj_units(ms):
                units = []

                def unit(m, nn):
                    def f():
                        msl = slice(m * 128, (m + 1) * 128)
                        os_ = slice(nn * 512, (nn + 1) * 512)
                        op = pj_ps.tile([128, 512], f32, tag="mi",
                                         name="opu")
                        for fc in range(4):
                            nc.tensor.matmul(
                                op[:, 0:512],
                                lhsT=at_sb[:, fc, msl],
                                rhs=wout_sb[:, fc, os_],
                                start=(fc == 0),
                                stop=(fc == 3),
                            )
                        ost = work.tile([128, 512], f32, tag="ost")
                        if ocopy == "dve":
                            nc.vector.tensor_copy(ost, op[:, 0:512])
                        else:
                            nc.scalar.copy(ost, op[:, 0:512])
                        nc.sync.dma_start(out=out_d[msl, os_], in_=ost)
                    return f

                for m in ms:
                    for nn in range(2):
                        units.append(unit(m, nn))
                return units

            from contextlib import nullcontext

            def _emit_all():
                if order == "inter":
                    emit_proj(0)
                    emit_kvaug(0)
                    prev_fin = []
                    for qb in range(NQB):
                        units = []
                        if qb + 1 < NQB:
                            units += proj_units(qb + 1)
                            units.append(lambda q=qb + 1: emit_kvaug(q))
                        if qb > 0:
                            units += outproj_units(
                                range(4 * (qb - 1), 4 * qb))
                        # [slots_remaining, unit list]: drained evenly;
                        # the previous pair's normalize tail goes first
                        fillers = [4 * (4 * qb + 4), units]
                        for pair in range(4):
                            fillers[1][0:0] = prev_fin
                            fins = emit_attn3(pair, qb, fillers)
                            if fin_inline:
                                for f in fins:
                                    f()
                                prev_fin = []
                            else:
                                prev_fin = list(fins)
                        for f in fillers[1]:
                            f()
                    for f in prev_fin:
                        f()
                    for m in range(12, 16):
                        emit_outproj(m)
                elif order == "phase3":
                    for qb in range(NQB):
                        emit_proj(qb)
                    for qb in range(NQB):
                        emit_kvaug(qb)
                    for qb in range(NQB):
                        for pair in range(4):
                            f1, f2 = emit_attn3(pair, qb)
                            f1()
                            f2()
                    for m in range(16):
                        emit_outproj(m)
                elif order == "phase2":
                    for qb in range(NQB):
                        emit_proj(qb)
                    for qb in range(NQB):
                        emit_kvaug(qb)
                    for pair in range(4):
                        for qb in range(NQB):
                            emit_attn2(pair, qb)
                    for m in range(16):
                        emit_outproj(m)
                elif order == "hybrid":
                    for qb in range(NQB):
                        emit_proj(qb)
                    for qb in range(NQB):
                        emit_kvaug(qb)
                    for qb in range(NQB):
                        for pair in range(4):
                            for parity in range(2):
                                emit_attn(pair, qb, parity)
                        for m in range(4 * qb, 4 * qb + 4):
                            emit_outproj(m)
                else:
                    for qb in range(NQB):
                        emit_proj(qb)
                    for qb in range(NQB):
                        emit_kvaug(qb)
                    for pair in range(4):
                        for qb in range(NQB):
                            for parity in range(2):
                                emit_attn(pair, qb, parity)
                    for m in range(16):
                        emit_outproj(m)

            if reps:
                with tc.For_i(0, reps, 1):
                    _emit_all()
            else:
                _emit_all()

    if recip_impl == "lnexp":
        # Reorder activation tables so the Exp-placements resolve to the set
        # that also holds Ln (natural_log_exp_and_others); the default order
        # makes Exp pick exp_and_others and thrash table loads between the
        # Ln and Exp activations in the softmax reciprocal.
        import concourse.bacc as _bacc
        _orig = _bacc.get_activation_tables

        def _reordered(arch):
            t = _orig(arch)
            pref = "natural_log_exp_and_others"
            if pref in t:
                out = {pref: t[pref]}
                out.update({k: v for k, v in t.items() if k != pref})
                return out
            return t

        _bacc.get_activation_tables = _reordered
        try:
            nc.finalize()
        finally:
            _bacc.get_activation_tables = _orig
    else:
        nc.finalize()
    return nc


def _host_prep(x, Wq, Wkv, null_kv, Wout):
    x = np.asarray(x, dtype=np.float32)
    Wq = np.asarray(Wq, dtype=np.float32)
    Wkv = np.asarray(Wkv, dtype=np.float32)
    null_kv = np.asarray(null_kv, dtype=np.float32)
    Wout = np.asarray(Wout, dtype=np.float32)

    scale = DH ** -0.5
    wq_scaled = (Wq * scale).astype(np.float16)
    wkv2 = np.concatenate([Wkv, Wkv], axis=1).astype(np.float16)  # [1024,128]
    nullkv2 = np.concatenate([null_kv, null_kv]).astype(np.float16).reshape(128, 1)
    wout16 = Wout.astype(np.float16)
    ident = np.eye(128, dtype=np.float16)

    # masks[t][i, j] = 1 if j >= i + 128*t - 1  (ST layout: i = k within chunk,
    # j = q within 512 block; delta = 128*t - 1 for diagonal chunk t)
    i_idx = np.arange(128)[:, None]
    j_idx = np.arange(QB)[None, :]
    masks = np.concatenate(
        [(j_idx >= i_idx + 128 * t - 1).astype(np.float16) for t in range(4)], axis=1
    )  # [128, 2048]

    in_maps = []
    for core in range(8):
        b, hg = core // 2, core % 2
        in_maps.append(
            {
                "xt": np.ascontiguousarray(x[b].T).astype(np.float16),
                "wq": wq_scaled[:, hg * 512:(hg + 1) * 512].copy(),
                "wkv2": wkv2,
                "nullkv2": nullkv2,
                "wout": np.ascontiguousarray(wout16[hg * 512:(hg + 1) * 512, :]),
                "masks": masks,
                "ident": ident,
            }
        )
    return in_maps


def kernel(x, Wq, Wkv, null_kv, Wout, _trace=False):
    from concourse import bass_utils

    if "nc" not in _PROGRAM_CACHE:
        _PROGRAM_CACHE["nc"] = _build_program()
    nc = _PROGRAM_CACHE["nc"]

    in_maps = _host_prep(x, Wq, Wkv, null_kv, Wout)
    res = bass_utils.run_bass_kernel_spmd(
        nc, in_maps, core_ids=list(range(8)), trace=_trace
    )
    _PROGRAM_CACHE["last_result"] = res

    outs = [np.asarray(r["out"], dtype=np.float32) for r in res.results]
    full = np.stack([outs[2 * b] + outs[2 * b + 1] for b in range(B)], axis=0)
    return full

